# revision 1
# baseline (speedup 1.0000x reference)
"""Trainium2 Bass kernel for the CapHLA_EL model (8-core data parallel).

Model per sample: X=[59,21] -> conv module (pointwise 21->6400, GLU,
depthwise 9-tap conv over positions, BN+silu, pointwise 3200->21),
residual+LN, 9-head self-attention (head_dim 21), residual+LN,
MLP head (1239->800 silu/bn ->64 relu ->2).

Strategy (per core, 128 samples, groups of 8):
 - channel-major layout [channels on partitions, (sample, position) free].
 - depthwise conv as 9 PSUM-accumulating PE matmuls with host-precomputed
   diagonal stationaries (BN scale folded in) over shifted windows of a
   zero-padded stride-67 sample layout.
 - GLU / BN-shift / silu via ACT sigmoids (per-partition bias APs) and DVE
   fused scalar_tensor_tensor ops.
 - biases folded into matmuls (ones-row trick) or ACT/STT scalars.
 - LayerNorm in transposed layout: sums via PE ones-matmuls, per-column
   scale via PE row-broadcast matmuls + DVE tensor_tensor.
 - attention: per-(sample,head) matmuls; exp without max-subtraction
   (scores are tiny for this input distribution); softmax denominator via
   a ones-column appended to V; division deferred to after the O-matmul.
"""

import math
import numpy as np
import ml_dtypes

BF16 = ml_dtypes.bfloat16

# ---------------- configuration ----------------
_FULL = dict(
    B=1024, N_CORES=8,
    L=59, V=21, C=3200, KT=9, H=9, NH=800,
    G=8,            # samples per group (matmul N = 8*59 = 472)
)
_EPS = 1e-5


def _cfg_derived(cfg):
    c = dict(cfg)
    c["SC"] = c["B"] // c["N_CORES"]          # samples per core
    c["NG"] = c["SC"] // c["G"]               # groups per core
    c["CH"] = c["C"] // 128                   # conv channel chunks
    c["CH2"] = 2 * c["CH"]                    # W1 output chunks
    c["NHC"] = (c["NH"] + 127) // 128         # head hidden chunks
    c["NHP"] = c["NHC"] * 128
    c["LP"] = c["L"] + c["KT"] - 1            # padded length (67)
    c["GW"] = c["G"] * c["L"]                 # group width (472)
    c["NPAIR"] = c["G"] // 2
    assert c["C"] % 128 == 0 and c["SC"] % c["G"] == 0
    return c


# ---------------- tile drain patch ----------------
# walrus TPB_CTRL supports at most 1 sync-wait on the final Tile drain;
# spread the excess waits over preceding sync-engine nops.
def _patch_tile():
    import concourse.tile as tile_mod
    import concourse.mybir as mybir
    from concourse.vector_clock import ScopedClock

    if getattr(tile_mod.TileContext, "_drain_patched", False):
        return

    # several walrus instruction encodings only fit 1-2 sync-waits;
    # excess waits are moved onto adjacent same-engine nops
    _WLIM = {"InstDrain": 1, "InstNoOp": 1, "InstMatmult": 1,
             "InstDMACopy": 99, "InstMemset": 1}

    def _split_excess_waits(nc):
        cnt = [0]
        for bb in nc.m.functions[0].blocks:
            insts = bb.instructions
            i = 0
            while i < len(insts):
                inst = insts[i]
                lim = _WLIM.get(type(inst).__name__, 2)
                si = inst.sync_info
                if si is None or not si.on_wait or len(si.on_wait) <= lim:
                    i += 1
                    continue
                extra, keep = si.on_wait[:-lim], si.on_wait[-lim:]
                si.on_wait = keep
                for j, w in enumerate(extra):
                    cnt[0] += 1
                    nop = mybir.InstNoOp(
                        name=f"I-wsplit-{cnt[0]}",
                        engine=inst.engine,
                        bass_nofuse=True,
                        sync_info=mybir.SyncInfo(on_wait=[w], on_update=[]),
                    )
                    insts.insert(i + j, nop)
                i += len(extra) + 1

    _patch_tile.split_excess_waits = _split_excess_waits

    def _drain_and_barrier(self, tick_clock, wait_clock):
        nops = [self.nc.sync.nop(nofuse=True, hint=f"drain_split_{i}")
                for i in range(32)]
        drain_inst = self.nc.sync.drain()
        wait_clock.add_sem_waits(
            drain_inst.ins, ScopedClock({None: tick_clock.global_clock})
        )
        si = drain_inst.ins.sync_info
        waits = list(si.on_wait or [])
        if len(waits) > 1:
            extra, keep = waits[:-1], waits[-1:]
            si.on_wait = keep
            for i, nop in enumerate(nops):
                chunk = extra[i:i + 1]
                if not chunk:
                    break
                if nop.ins.sync_info is None:
                    nop.ins.sync_info = mybir.SyncInfo(on_wait=chunk, on_update=[])
                else:
                    nop.ins.sync_info.on_wait = chunk

        self.nc.all_engine_barrier()
        assert self.sems is not None
        popped = self.nc._tile_sem_poison_stack.pop()
        assert popped is self._sem_poison
        self.nc.clear_and_free_semaphores(list(self.sems.allocated().values()))
        self.nc.all_engine_barrier()

    tile_mod.TileContext._drain_and_barrier = _drain_and_barrier
    tile_mod.TileContext._drain_patched = True


# ---------------- host-side weight preparation ----------------
def _prep_weights(inp, c):
    """Transform reference weights into device layouts (numpy, shared by all cores)."""
    f32 = np.float32
    V, L, C, KT, H, NH = c["V"], c["L"], c["C"], c["KT"], c["H"], c["NH"]
    CH, CH2, NHC, NHP = c["CH"], c["CH2"], c["NHC"], c["NHP"]

    W1 = np.asarray(inp["W1"], f32)        # [2C, V]
    b1 = np.asarray(inp["b1"], f32)        # [2C]
    Wd = np.asarray(inp["Wd"], f32)[:, 0, :]   # [C, KT]
    bd = np.asarray(inp["bd"], f32)
    bn1_g = np.asarray(inp["bn1_g"], f32); bn1_b = np.asarray(inp["bn1_b"], f32)
    bn1_m = np.asarray(inp["bn1_m"], f32); bn1_v = np.asarray(inp["bn1_v"], f32)
    W2 = np.asarray(inp["W2"], f32)        # [V, C]
    b2 = np.asarray(inp["b2"], f32)
    Wq = np.asarray(inp["Wq"], f32); Wk = np.asarray(inp["Wk"], f32)
    Wv = np.asarray(inp["Wv"], f32); Wo = np.asarray(inp["Wo"], f32)
    Wf1 = np.asarray(inp["Wf1"], f32)      # [L*V, NH]
    bf1 = np.asarray(inp["bf1"], f32)
    bn2_g = np.asarray(inp["bn2_g"], f32); bn2_b = np.asarray(inp["bn2_b"], f32)
    bn2_m = np.asarray(inp["bn2_m"], f32); bn2_v = np.asarray(inp["bn2_v"], f32)
    Wf2 = np.asarray(inp["Wf2"], f32); bf2 = np.asarray(inp["bf2"], f32)
    Wf3 = np.asarray(inp["Wf3"], f32); bf3 = np.asarray(inp["bf3"], f32)

    w = {}
    # W1^T with bias row: [V+1, 2C]
    w["w1t"] = np.concatenate([W1.T, b1[None, :]], axis=0).astype(BF16)

    # depthwise conv: fold BN scale into weights, BN shift (+bd) into bias.
    s1 = bn1_g / np.sqrt(bn1_v + _EPS)                   # [C]
    t1 = (bd - bn1_m) * s1 + bn1_b                       # [C]
    Wds = Wd * s1[:, None]                               # [C, KT]
    diag = np.zeros((128, CH, KT, 128), f32)
    idx = np.arange(128)
    for i in range(CH):
        for k in range(KT):
            diag[idx, i, k, idx] = Wds[i * 128 + idx, k]
    w["diag"] = diag.reshape(128, CH * KT * 128).astype(BF16)
    w["bnt"] = t1.reshape(CH, 128).T.astype(f32).copy()  # [128, CH]

    # W2^T chunks: [128, CH*V]; w2t[c,(i,o)] = W2[o, i*128+c]
    w["w2t"] = np.ascontiguousarray(
        W2.T.reshape(CH, 128, V).transpose(1, 0, 2).reshape(128, CH * V)
    ).astype(BF16)
    w["b2"] = b2.reshape(V, 1).astype(f32)

    # qkv weights with heads padded to 32-partition stride:
    # head h lives at columns 32*h .. 32*h+21 (zeros elsewhere)
    def padheads(W):
        Wp = np.zeros((V, H * 32), f32)
        for h in range(H):
            Wp[:, 32 * h:32 * h + V] = W[:, V * h:V * h + V]
        return Wp.astype(BF16)
    w["wq"] = padheads(Wq); w["wk"] = padheads(Wk); w["wv"] = padheads(Wv)
    w["wo6"] = Wo[:126].astype(BF16); w["wo3"] = Wo[126:].astype(BF16)

    # head: wf1 arranged [L, V*NHC*128]: [t, (v, cc, m)] = Wf1[t*V+v, cc*128+m] + pad
    Wf1p = np.zeros((L * V, NHP), f32); Wf1p[:, :NH] = Wf1
    w["wf1"] = np.ascontiguousarray(
        Wf1p.reshape(L, V, NHC, 128).reshape(L, V * NHC * 128)
    ).astype(BF16)
    # bf1 folded via bn2: y2 = silu(h1+bf1)... NOTE: reference adds bf1 BEFORE
    # silu: h = silu(X@Wf1 + bf1). Fold bf1 into the sigmoid bias + STT scalar.
    bf1p = np.zeros((NHP,), f32); bf1p[:NH] = bf1
    w["bf1"] = bf1p.reshape(NHC, 128).T.astype(f32).copy()   # [128, NHC]
    # bn2 folded into Wf2 (scale) and bf2 (shift):
    s2 = bn2_g / np.sqrt(bn2_v + _EPS)
    t2 = bn2_b - bn2_m * s2
    Wf2p = np.zeros((NHP, 64), f32); Wf2p[:NH] = Wf2 * s2[:, None]
    w["wf2"] = np.ascontiguousarray(
        Wf2p.reshape(NHC, 128, 64).transpose(1, 0, 2).reshape(128, NHC * 64)
    ).astype(BF16)
    w["bf2p"] = (bf2 + t2 @ Wf2).reshape(64, 1).astype(f32)
    w["wf3"] = Wf3.astype(BF16)
    w["bf3"] = bf3.reshape(2, 1).astype(f32)

    w["idf"] = np.eye(128, dtype=f32)
    w["idb"] = np.eye(128, dtype=f32).astype(BF16)
    w["onesc"] = np.ones((V, 1), f32).astype(BF16)
    w["onesr"] = np.ones((1, V), f32)
    return w


# ---------------- device program ----------------
def _build(c):
    import concourse.bass as bass
    import concourse.bacc as bacc
    import concourse.mybir as mybir
    from concourse.tile import TileContext
    from contextlib import ExitStack

    _patch_tile()

    f32 = mybir.dt.float32
    bf16 = mybir.dt.bfloat16
    AL = mybir.AluOpType
    AF = mybir.ActivationFunctionType

    V, L, C, KT, H = c["V"], c["L"], c["C"], c["KT"], c["H"]
    CH, CH2, NHC = c["CH"], c["CH2"], c["NHC"]
    SC, NG, G, GW, LP, NPAIR = c["SC"], c["NG"], c["G"], c["GW"], c["LP"], c["NPAIR"]
    PAD = (KT - 1) // 2          # 4
    SINV = 1.0 / math.sqrt(V)    # attention scale

    nc = bacc.Bacc()
    P = lambda n, s, d: nc.declare_dram_parameter(n, s, d, isOutput=False)
    xin = P("xin", [SC * L, V], f32)
    w1t = P("w1t", [V + 1, CH2 * 128], bf16)
    diag = P("diag", [128, CH * KT * 128], bf16)
    bnt = P("bnt", [128, CH], f32)
    w2t = P("w2t", [128, CH * V], bf16)
    b2 = P("b2", [V, 1], f32)
    wq = P("wq", [V, H * 32], bf16)
    wk = P("wk", [V, H * 32], bf16)
    wv = P("wv", [V, H * 32], bf16)
    wo6 = P("wo6", [126, V], bf16)
    wo3 = P("wo3", [63, V], bf16)
    wf1 = P("wf1", [L, V * NHC * 128], bf16)
    bf1 = P("bf1", [128, NHC], f32)
    wf2 = P("wf2", [128, NHC * 64], bf16)
    bf2p = P("bf2p", [64, 1], f32)
    wf3 = P("wf3", [64, 2], bf16)
    bf3 = P("bf3", [2, 1], f32)
    idf = P("idf", [128, 128], f32)
    idb = P("idb", [128, 128], bf16)
    onesc = P("onesc", [V, 1], bf16)
    onesr = P("onesr", [1, V], f32)
    out = nc.declare_dram_parameter("out", [2, 2, SC // 2], f32, isOutput=True)
    dbg_tensors = {}

    def dbg(name, ap):
        if not c.get("DBG"):
            return
        t = nc.declare_dram_parameter("dbg_" + name, list(ap.shape),
                                      ap.dtype, isOutput=True)
        nc.sync.dma_start(out=t[:], in_=ap)
        dbg_tensors[name] = t

    with TileContext(nc) as tc, ExitStack() as ctx:
        wp = ctx.enter_context(tc.tile_pool(name="wp", bufs=1))
        sb2 = ctx.enter_context(tc.tile_pool(name="sb2", bufs=2))   # group-lifetime
        sb3 = ctx.enter_context(tc.tile_pool(name="sb3", bufs=3))   # chunk-transient
        sbl = ctx.enter_context(tc.tile_pool(name="sbl", bufs=2))   # LN row scratch
        pp = ctx.enter_context(tc.tile_pool(name="pp", bufs=1, space="PSUM"))

        class _PsumPool:
            def __init__(self, tag, bufs):
                self.tag, self.bufs = tag, bufs
            def tile(self, shape, dt, tag=None):
                return pp.tile(shape, dt, tag=self.tag, bufs=self.bufs,
                               name=tag or self.tag)

        psA = _PsumPool("pag", 2)      # W1 a/g outputs (shared pair slots)
        psD = _PsumPool("pd", 1)       # depthwise accumulator
        psW = _PsumPool("acc21", 2)    # pw2 / pao group accumulators
        psX = _PsumPool("pxt", 1)      # input transpose
        psT = _PsumPool("pscr", 2)     # everything else

        # ---- load weights into SBUF ----
        def wload(dram, shape, dt, name):
            t = wp.tile(shape, dt, tag=name)
            nc.sync.dma_start(out=t[:], in_=dram[:])
            return t

        w1t_s = wload(w1t, [V + 1, CH2 * 128], bf16, "w1t")
        diag_s = wp.tile([128, CH * KT * 128], bf16, tag="diag")
        # split the big diag DMA so several queues run in parallel
        dsplit = max(1, CH // 4)
        for j in range(0, CH, dsplit):
            cols = slice(j * KT * 128, min(CH, j + dsplit) * KT * 128)
            nc.sync.dma_start(out=diag_s[:, cols], in_=diag[:, cols])
        bnt_s = wload(bnt, [128, CH], f32, "bnt")
        w2t_s = wload(w2t, [128, CH * V], bf16, "w2t")
        b2_s = wload(b2, [V, 1], f32, "b2")
        wq_s = wload(wq, [V, H * 32], bf16, "wq")
        wk_s = wload(wk, [V, H * 32], bf16, "wk")
        wv_s = wload(wv, [V, H * 32], bf16, "wv")
        wo6_s = wload(wo6, [126, V], bf16, "wo6")
        wo3_s = wload(wo3, [63, V], bf16, "wo3")
        wf1_s = wload(wf1, [L, V * NHC * 128], bf16, "wf1")
        bf1_s = wload(bf1, [128, NHC], f32, "bf1")
        wf2_s = wload(wf2, [128, NHC * 64], bf16, "wf2")
        bf2p_s = wload(bf2p, [64, 1], f32, "bf2p")
        wf3_s = wload(wf3, [64, 2], bf16, "wf3")
        bf3_s = wload(bf3, [2, 1], f32, "bf3")
        idf_s = wload(idf, [128, 128], f32, "idf")
        idb_s = wload(idb, [128, 128], bf16, "idb")
        onesc_s = wload(onesc, [V, 1], bf16, "onesc")
        onesr_s = wload(onesr, [1, V], f32, "onesr")

        # persistent rings (pads / ones set once)
        u_ring = []
        for r in range(3):
            t = wp.tile([128, G, LP], bf16, tag=f"uring{r}")
            nc.vector.memset(t[:, :, 0:PAD], 0.0)
            nc.vector.memset(t[:, :, PAD + L:LP], 0.0)
            u_ring.append(t)
        vtm_ring = []
        for r in range(3):
            pair = []
            for si in range(2):
                t = wp.tile([L, H, 22], bf16, tag=f"vtmring{r}_{si}")
                nc.vector.memset(t[:, :, 21:22], 1.0)
                pair.append(t)
            vtm_ring.append(pair)
        # token-major LN2 output, even/odd samples of each pair split so the
        # head matmul rhs starts at partition 0
        x3tm0 = wp.tile([L, SC // 2, V], bf16, tag="x3tm0")
        x3tm1 = wp.tile([L, SC // 2, V], bf16, tag="x3tm1")

        uc = [0]   # u ring counter
        vc = [0]   # vtm ring counter

        def mm(o, l, r, **kw):
            nc.tensor.matmul(o, l, r, **kw)

        # ---- layernorm in transposed layout ----
        def ln_t(xp, xn_tag):
            """xp: [V, GW] bf16 pre-LN -> returns [V, GW] bf16 normalized."""
            sq = sb3.tile([V, GW], bf16, tag="ln_sq")
            nc.scalar.activation(sq[:], xp[:], AF.Square)
            pst0 = psT.tile([1, GW], f32, tag="ln_pst0")
            pst1 = psT.tile([1, GW], f32, tag="ln_pst1")
            mm(pst0[:], onesc_s[:], xp[:])
            mm(pst1[:], onesc_s[:], sq[:])
            mu = sbl.tile([1, GW], f32, tag="ln_mu")
            nc.vector.tensor_scalar_mul(mu[:], pst0[:], 1.0 / V)
            t_ = sbl.tile([1, GW], f32, tag="ln_t")
            nc.vector.tensor_tensor(t_[:], mu[:], mu[:], AL.mult)
            nc.vector.scalar_tensor_tensor(
                t_[:], pst1[:], 1.0 / V, t_[:], AL.mult, AL.subtract)
            nc.vector.tensor_scalar_add(t_[:], t_[:], _EPS)
            nc.vector.reciprocal(t_[:], t_[:])
            row_r = sbl.tile([1, GW], f32, tag="ln_row_r")
            row_n = sbl.tile([1, GW], f32, tag="ln_row_n")
            nc.scalar.activation(row_r[:], t_[:], AF.Sqrt)
            nc.vector.scalar_tensor_tensor(
                row_n[:], mu[:], -1.0, row_r[:], AL.mult, AL.mult)
            pbR = psT.tile([V, GW], f32, tag="ln_pbR")
            pbN = psT.tile([V, GW], f32, tag="ln_pbN")
            mm(pbR[:], onesr_s[:], row_r[:])
            mm(pbN[:], onesr_s[:], row_n[:])
            t1 = sb3.tile([V, GW], bf16, tag="ln_t1")
            nc.vector.tensor_tensor(t1[:], xp[:], pbR[:], AL.mult)
            xn = sb2.tile([V, GW], bf16, tag=xn_tag)
            nc.vector.tensor_tensor(xn[:], t1[:], pbN[:], AL.add)
            return xn

        # ================= per-group pipeline =================
        for g in range(NG):
            s0 = g * G
            # ---- input: token-major load + transpose to XT [V+1, GW] ----
            pxt = psX.tile([V + 1, GW], f32, tag="pxt")
            for p in range(NPAIR):
                tmp = sb3.tile([2 * L, V + 1], f32, tag="in_tm")
                nc.sync.dma_start(
                    out=tmp[:, 0:V],
                    in_=xin[(s0 + 2 * p) * L:(s0 + 2 * p + 2) * L, :])
                nc.vector.memset(tmp[:, V:V + 1], 1.0)
                nc.tensor.transpose(
                    pxt[:, p * 2 * L:(p + 1) * 2 * L], tmp[:], idf_s[0:2 * L, 0:2 * L])
            xt = sb2.tile([V + 1, GW], bf16, tag="xt")
            nc.scalar.copy(xt[:], pxt[:])
            if g == 0:
                dbg("xt", xt[:])

            # ---- conv module ----
            pw2 = psW.tile([V, GW], f32, tag="pw2")
            for i in range(CH):
                pa = psA.tile([128, G, L], f32, tag="pa")
                pg = psA.tile([128, G, L], f32, tag="pg")
                mm(pa[:], w1t_s[:, i * 128:(i + 1) * 128], xt[:])
                mm(pg[:], w1t_s[:, (CH + i) * 128:(CH + i + 1) * 128], xt[:])
                sg = sb3.tile([128, G, L], bf16, tag="sg")
                nc.scalar.activation(sg[:], pg[:], AF.Sigmoid)
                u = u_ring[uc[0] % 3]; uc[0] += 1
                nc.vector.tensor_tensor(u[:, :, PAD:PAD + L], pa[:], sg[:], AL.mult)
                pd = psD.tile([128, G, L], f32, tag="pd")
                for k in range(KT):
                    mm(pd[:],
                       diag_s[:, (i * KT + k) * 128:(i * KT + k + 1) * 128],
                       u[:, :, k:k + L],
                       start=(k == 0), stop=(k == KT - 1))
                sz = sb3.tile([128, G, L], bf16, tag="sz")
                nc.scalar.activation(sz[:], pd[:], AF.Sigmoid,
                                     bias=bnt_s[:, i:i + 1], scale=1.0)
                y = sb3.tile([128, G, L], bf16, tag="y")
                nc.vector.scalar_tensor_tensor(
                    y[:], pd[:], bnt_s[:, i:i + 1], sz[:], AL.add, AL.mult)
                mm(pw2[:], w2t_s[:, i * V:(i + 1) * V], y[:],
                   start=(i == 0), stop=(i == CH - 1))

            # residual + b2 -> pre-LN1
            x2p = sb3.tile([V, GW], bf16, tag="x2p")
            nc.vector.scalar_tensor_tensor(
                x2p[:], pw2[:], b2_s[:], xt[0:V, :], AL.add, AL.add)
            if g == 0:
                dbg("x2p", x2p[:])
            x2 = ln_t(x2p, "x2")
            if g == 0:
                dbg("x2", x2[:])
            if c.get("STOP_AFTER") == "conv":
                if g == NG - 1:
                    nc.gpsimd.dma_start(out=out[:], in_=x2[0:2, 0:2 * (SC // 2)])
                continue

            # ---- QKV: per-head matmuls (stationary/moving must sit at
            # partition base 0 — non-zero row bases crash the PE here),
            # packed per-head along the free dim: [V, H, GW] ----
            def qkv(w_s, tag):
                t = sb2.tile([V, H, GW], bf16, tag=tag, bufs=1)
                for h in range(H):
                    pq = psT.tile([V, GW], f32, tag="qkvp")
                    mm(pq[:], w_s[:, 32 * h:32 * h + V], x2[:])
                    nc.scalar.copy(t[:, h, :], pq[:])
                return t

            qh = qkv(wq_s, "qh")
            kh = qkv(wk_s, "kh")
            vh = qkv(wv_s, "vh")
            if c.get("STOP_AFTER") == "qkv":
                if g == NG - 1:
                    nc.gpsimd.dma_start(out=out[:], in_=qh[0:2, 0, 0:2 * (SC // 2)])
                continue

            def hslice(ts, h, cols):
                return ts[:, h, cols]

            _ATT_LVL = {"pv": 0, "scores": 1, "exp": 2, "omm": 3, "recip": 4,
                        "osc": 5, "ot": 6}.get(c.get("STOP_AFTER"), 99)
            pao = psW.tile([V, GW], f32, tag="pao")
            for p in range(NPAIR):
                col = slice(p * 2 * L, (p + 1) * 2 * L)
                # v -> token-major [123, H, 22]; pair halves at partitions 0/64
                pv = psT.tile([123, H, 22], bf16, tag="pv")
                for h in range(H):
                    for si in range(2):
                        scol = slice((p * 2 + si) * L, (p * 2 + si + 1) * L)
                        nc.tensor.matmul(
                            pv[64 * si:64 * si + L, h, 0:V],
                            hslice(vh, h, scol), idb_s[0:V, 0:V],
                            is_transpose=True, skip_group_check=(si == 1))
                vtm = vtm_ring[vc[0] % 3]; vc[0] += 1
                for si in range(2):
                    sl = slice(64 * si, 64 * si + L)
                    nc.scalar.copy(vtm[si][:, :, 0:V], pv[sl, :, 0:V])
                if _ATT_LVL < 1:
                    continue
                # scores^T = k^T-stationary @ q^T-moving  -> [k-pos, q-pos]
                ps_ = psT.tile([123, 8, L], f32, tag="ps")
                ps8 = psT.tile([123, L], f32, tag="ps8")
                for h in range(c.get("SC_H", H)):
                    for si in range(c.get("SC_SI", 2)):
                        scol = slice((p * 2 + si) * L, (p * 2 + si + 1) * L)
                        o = (ps_[64 * si:64 * si + L, h, :] if h < 8
                             else ps8[64 * si:64 * si + L, :])
                        mm(o, hslice(kh, h, scol), hslice(qh, h, scol),
                           skip_group_check=(si == 1))
                if _ATT_LVL < 2:
                    continue
                e_ = [sb3.tile([L, 8, L], bf16, tag=f"e{si}", name=f"e{si}")
                      for si in range(2)]
                e8 = [sb3.tile([L, L], bf16, tag=f"e8{si}", name=f"e8{si}")
                      for si in range(2)]
                for si in range(2):
                    sl = slice(64 * si, 64 * si + L)
                    nc.scalar.activation(e_[si][:], ps_[sl], AF.Exp, scale=SINV)
                    nc.scalar.activation(e8[si][:], ps8[sl], AF.Exp, scale=SINV)
                if _ATT_LVL < 3:
                    continue
                # o = exp-scores^T.T @ [v|1]
                # inner pitch 24 keeps base-64 slices bank-aligned for the sim
                po = psT.tile([123, H, 24], f32, tag="po")
                for h in range(H):
                    for si in range(2):
                        sl = slice(64 * si, 64 * si + L)
                        lhs = e_[si][:, h, :] if h < 8 else e8[si][:]
                        mm(po[sl, h, 0:22], lhs, vtm[si][:, h, :],
                           skip_group_check=(si == 1))
                if _ATT_LVL < 4:
                    continue
                r_ = sb3.tile([123, H, 1], f32, tag="r")
                osc = sb3.tile([123, H, V], bf16, tag="osc")
                for si in range(2):
                    sl = slice(64 * si, 64 * si + L)
                    nc.vector.reciprocal(r_[sl], po[sl, :, 21:22])
                if _ATT_LVL < 5:
                    continue
                # per-head tensor_scalar: the reciprocal is a per-partition
                # scalar for a fixed head (stride-0 broadcasts crash DVE here)
                for si in range(2):
                    sl = slice(64 * si, 64 * si + L)
                    for h in range(H):
                        nc.vector.tensor_scalar_mul(
                            osc[sl, h, :], po[sl, h, 0:V], r_[sl, h, :])
                # o^T then Wo
                # halves at 60-column pitch keep bf16 psum writes 4B-aligned
                pot6 = psT.tile([126, 2, 60], bf16, tag="pot6")
                pot3 = psT.tile([63, 2, 60], bf16, tag="pot3")
                for si in range(2):
                    sl = slice(64 * si, 64 * si + L)
                    nc.tensor.transpose(
                        pot6[:, si, 0:L], osc[sl, 0:6, :],
                        idb_s[64 * si:64 * si + L, 64 * si:64 * si + L])
                    nc.tensor.transpose(
                        pot3[:, si, 0:L], osc[sl, 6:9, :],
                        idb_s[64 * si:64 * si + L, 64 * si:64 * si + L])
                ot6 = sb3.tile([126, 2, L], bf16, tag="ot6")
                ot3 = sb3.tile([63, 2, L], bf16, tag="ot3")
                nc.scalar.copy(ot6[:], pot6[:, :, 0:L])
                nc.scalar.copy(ot3[:], pot3[:, :, 0:L])
                if g == 0 and p == 0:
                    for _si in range(2):
                        _sl = slice(64 * _si, 64 * _si + L)
                        dbg(f"e_{_si}", e_[_si][:]); dbg(f"e8_{_si}", e8[_si][:])
                        dbg(f"osc{_si}", osc[_sl])
                    dbg("ot6", ot6[:])
                if _ATT_LVL < 7:
                    continue
                mm(pao[:, col], wo6_s[:], ot6[:], start=True, stop=False,
                   skip_group_check=(p >= 1))
                mm(pao[:, col], wo3_s[:], ot3[:], start=False, stop=True,
                   skip_group_check=(p >= 1))

            if _ATT_LVL < 99:
                if g == NG - 1:
                    nc.gpsimd.dma_start(out=out[:], in_=x2[0:2, 0:2 * (SC // 2)])
                continue
            # residual + LN2
            x3p = sb3.tile([V, GW], bf16, tag="x3p")
            nc.vector.tensor_tensor(x3p[:], pao[:], x2[:], AL.add)
            x3 = ln_t(x3p, "x3")
            if g == 0:
                dbg("x3p", x3p[:]); dbg("x3", x3[:])
            if c.get("STOP_AFTER") == "attn":
                if g == NG - 1:
                    nc.gpsimd.dma_start(out=out[:], in_=x3[0:2, 0:2 * (SC // 2)])
                continue
            # token-major for the head (even/odd samples in separate tiles)
            for p in range(NPAIR):
                for si, x3tm in ((0, x3tm0), (1, x3tm1)):
                    px3 = psT.tile([L, V], bf16, tag="px3")
                    scol = slice((p * 2 + si) * L, (p * 2 + si + 1) * L)
                    nc.tensor.transpose(px3[:], x3[:, scol], idb_s[0:V, 0:V])
                    nc.scalar.copy(x3tm[:, g * NPAIR + p, :], px3[:])

        # ================= head =================
        if not c.get("STOP_AFTER"):
            sh_tiles = []
            for cc in range(NHC):
                # separate psum tiles (separate banks): start=True marks the whole
                # 2KB zero-region, so two groups must not share a bank
                ph_e = psT.tile([128, SC // 2], f32, tag="ph_e")
                ph_o = psT.tile([128, SC // 2], f32, tag="ph_o")
                for v in range(V):
                    l_ = wf1_s[:, (v * NHC + cc) * 128:(v * NHC + cc + 1) * 128]
                    mm(ph_e[:], l_, x3tm0[:, :, v:v + 1],
                       start=(v == 0), stop=(v == V - 1))
                    mm(ph_o[:], l_, x3tm1[:, :, v:v + 1],
                       start=(v == 0), stop=(v == V - 1))
                sgh = sb3.tile([128, SC], bf16, tag="sgh")
                sh = wp.tile([128, SC], bf16, tag=f"sh{cc}")
                for half, phh in ((0, ph_e), (1, ph_o)):
                    cols = slice(half * (SC // 2), (half + 1) * (SC // 2))
                    nc.scalar.activation(sgh[:, cols], phh[:], AF.Sigmoid,
                                         bias=bf1_s[:, cc:cc + 1], scale=1.0)
                    nc.vector.scalar_tensor_tensor(
                        sh[:, cols], phh[:], bf1_s[:, cc:cc + 1], sgh[:, cols],
                        AL.add, AL.mult)
                sh_tiles.append(sh)
            ph3 = psT.tile([64, SC], f32, tag="ph3")
            for cc in range(NHC):
                mm(ph3[:], wf2_s[:, cc * 64:(cc + 1) * 64], sh_tiles[cc][:],
                   start=(cc == 0), stop=(cc == NHC - 1))
            dbg("sh0", sh_tiles[0][:])
            dbg("x3tm0", x3tm0[:])
            rh = sb3.tile([64, SC], bf16, tag="rh")
            nc.scalar.activation(rh[:], ph3[:], AF.Relu,
                                 bias=bf2p_s[:], scale=1.0)
            dbg("rh", rh[:])
            pf = psT.tile([2, SC], f32, tag="pf")
            mm(pf[:], wf3_s[:], rh[:])
            outt = sb2.tile([2, 2, SC // 2], f32, tag="outt")
            nc.vector.tensor_scalar_add(outt[:], pf[:], bf3_s[:])
            nc.sync.dma_start(out=out[:], in_=outt[:])

    return nc


# ---------------- per-core input maps ----------------
def _in_maps(inp, c):
    w = _prep_weights(inp, c)
    pep = np.asarray(inp["pep"], np.float32)
    mhc = np.asarray(inp["mhc"], np.float32)
    X = np.concatenate([pep, mhc], axis=1)          # [B, L, V]
    SC = c["SC"]
    maps = []
    for i in range(c["N_CORES"]):
        m = dict(w)
        m["xin"] = np.ascontiguousarray(
            X[i * SC:(i + 1) * SC].reshape(SC * c["L"], c["V"]))
        maps.append(m)
    return maps


def _gather(results, c):
    SC = c["SC"]
    outs = []
    for r in results:
        o = np.asarray(r["out"])                    # [2, 2, SC//2]
        full = np.empty((SC, 2), np.float32)
        full[0::2] = o[:, 0, :].T
        full[1::2] = o[:, 1, :].T
        outs.append(full)
    return np.concatenate(outs, axis=0)


_BUILT = {}


def _get_program(c):
    key = tuple(sorted((k, v) for k, v in c.items()))
    if key not in _BUILT:
        _BUILT[key] = _build(c)
    return _BUILT[key]


def run(inputs, cfg=None, trace=False):
    from concourse.bass_utils import run_bass_kernel_spmd
    c = _cfg_derived(cfg or _FULL)
    nc = _get_program(c)
    if not getattr(nc, "_finalized_for_hw", False):
        nc.finalize()
        nc._finalized_for_hw = True
    maps = _in_maps(inputs, c)
    res = run_bass_kernel_spmd(nc, maps, list(range(c["N_CORES"])), trace=trace)
    return _gather(res.results, c), res


def kernel(**inputs):
    out, _ = run(inputs)
    return out

